# revision 8
# baseline (speedup 1.0000x reference)
"""ChannelAttentionPropagation1D kernel for 8x TRN2 NeuronCores.

Reference computation (per batch b):
  kv[c,d]   = sum_{t,n} key_mem[b,t,n,c] * val_mem[b,t,n,d]    # (64, 64)
  kv_soft   = softmax(kv, axis=c)
  out[n,d]  = alpha * (key_cur[b] @ kv_soft)[n,d] + val_cur[b,n,d]

Sharding (8 cores): batch-pair. Core c owns batch b = c//2, token-half
h = c%2. Phase 1 contracts its 65536-token half of key_mem/val_mem into
a partial kvT; ONE 2-rank AllGather (cores 2b <-> 2b+1) exchanges the
32KB PSUM partials; both cores reduce + softmax locally. Phase 2
computes the core's 8192-token slice of the output.

Precision: the kv softmax is extremely sharp (top-2 logit gap ~500) and
alpha is small, so fp16 inputs to both matmuls keep the final rel-fro
error ~2e-4, far under the 2e-2 gate, while halving HBM traffic. The
host casts all inputs to fp16; PSUM accumulation stays fp32 and the
output is stored fp32.

Layouts:
  - phase 1 accumulates kvT[d,c] in PSUM with two alternating PE column
    groups; the raw [128,64] PSUM partial is DMAd straight to the
    collective input (no pre-reduction), and the fold happens after the
    gather with two DVE adds.
  - phase 2 is computed TRANSPOSED and FUSED: the stationary operand is
    W = [kv_soft[c,d]; I64], and the moving operand stacks
    [alpha*key_curT; val_curT] on the 128 partitions, so one matmul per
    512-token tile yields outT[d,tok] = kv_soft^T@key_curT + val_curT
    directly in PSUM (no DVE adds); stores go PSUM -> DRAM. Two PE
    column groups process the core's two 4096-token groups concurrently.
    The host transposes the returned [128, 4096] block back to
    [8192, 64].
"""

import numpy as np

import concourse.bacc as bacc
import concourse.mybir as mybir
import concourse.tile as tile
from concourse import bass_utils, masks

F32 = mybir.dt.float32
F16 = mybir.dt.float16

N_CORES = 8
N, T, NTOK, C, C2 = 4, 8, 16384, 64, 64
TOK_ALL = T * NTOK          # 131072 tokens per batch
TOK_HALF = TOK_ALL // 2     # 65536 phase-1 tokens per core
NSL = NTOK // 2             # 8192 phase-2 output tokens per core
CHUNK = 8192                # phase-1 tokens per DMA chunk
N_CHUNKS = TOK_HALF // CHUNK    # 8
A_TILES = CHUNK // 128      # 64 matmul token-tiles per chunk
G_TILES = 8                 # phase-2: 8 psum tiles of 2x512 tokens

_CACHE = {}

# Extra kwargs forwarded to run_bass_kernel_spmd (used by the profiling
# harness to request an NTFF trace; empty for normal correctness runs).
_RUN_OPTS = {}


def _build_program():
    nc = bacc.Bacc(
        "TRN2",
        target_bir_lowering=False,
        debug=False,
        enable_asserts=False,
        num_devices=N_CORES,
    )

    km = nc.dram_tensor("key_mem", [TOK_HALF, C], F16, kind="ExternalInput").ap()
    vm = nc.dram_tensor("val_mem", [TOK_HALF, C2], F16, kind="ExternalInput").ap()
    # kvc: [128, 8192] fp16. Rows 0:64 = alpha*key_cur^T (channels c),
    # rows 64:128 = val_cur^T (channels d). Cols 0:4096 = token group A
    # (slice tokens 0:4096), cols 4096:8192 = group B (4096:8192).
    kvc = nc.dram_tensor("kvc", [128, NSL], F16, kind="ExternalInput").ap()
    # outT: rows 0:64 = out[d, tok] for group A, rows 64:128 for group B.
    out = nc.dram_tensor("outT", [128, NSL // 2], F32, kind="ExternalOutput").ap()

    with tile.TileContext(nc) as tc:
        with (
            tc.tile_pool(name="persist", bufs=1) as persist,
            tc.tile_pool(name="big", bufs=4) as big,
            tc.tile_pool(name="tmp", bufs=1) as tmp,
            tc.tile_pool(name="ps", bufs=1, space="PSUM") as ps,
            tc.tile_pool(name="po", bufs=4, space="PSUM") as po,
            tc.tile_pool(name="dram", bufs=1, space="DRAM") as dram,
        ):
            ident = persist.tile([128, 128], F32)
            masks.make_identity(nc, ident[:])

            kvc_sb = persist.tile([128, NSL], F16)
            # stationary phase-2 weights: rows 0:64 = kv_soft[c, d],
            # rows 64:128 = I64 (the val_cur passthrough).
            w_sb = persist.tile([128, C2], F16)
            nc.vector.tensor_copy(w_sb[64:128, :], ident[0:64, 0:64])

            rb = persist.tile([C2, 4 * C], F32)
            kvt_red = persist.tile([C2, C], F32)

            # ---- phase 1: partial kvT[d, c], col-tiled 2x ----
            kv_ps = ps.tile([128, C], F32)
            for ch in range(N_CHUNKS):
                k_sb = big.tile([128, CHUNK // 128 * C], F16, tag="k")
                v_sb = big.tile([128, CHUNK // 128 * C2], F16, tag="v")
                sl = slice(ch * CHUNK, (ch + 1) * CHUNK)
                nc.sync.dma_start(
                    k_sb[:], km[sl, :].rearrange("(p a) c -> p (a c)", p=128)
                )
                nc.sync.dma_start(
                    v_sb[:], vm[sl, :].rearrange("(p a) c -> p (a c)", p=128)
                )
                if ch == 1:
                    # phase-2 inputs ride the scalar (ACT) DMA FIFO so they
                    # never delay the phase-1 chunk stream on the sync FIFO.
                    nc.scalar.dma_start(kvc_sb[:], kvc)
                for a in range(A_TILES):
                    half = a % 2
                    nc.tensor.matmul(
                        kv_ps[64 * half:64 * half + C2, :],
                        lhsT=v_sb[:, a * C2:(a + 1) * C2],
                        rhs=k_sb[:, a * C:(a + 1) * C],
                        start=(ch == 0 and a < 2),
                        stop=(ch == N_CHUNKS - 1 and a >= A_TILES - 2),
                        tile_position=(0, 64 * half),
                    )

            # ---- pairwise exchange: 2-rank AllGather of the raw PSUM
            # partial (both col-group halves, 32KB) straight from PSUM.
            ar_in = dram.tile([128, C], F32, tag="ar_in", name="ar_in")
            ar_out = dram.tile([2, 128, C], F32, tag="ar_out", name="ar_out")
            kvt128 = persist.tile([128, C], F32)
            nc.vector.tensor_copy(kvt128[:], kv_ps[:])
            nc.scalar.dma_start(ar_in[:], kvt128[:])
            nc.gpsimd.collective_compute(
                "AllGather",
                mybir.AluOpType.bypass,
                replica_groups=[[2 * i, 2 * i + 1] for i in range(4)],
                ins=[ar_in.opt()],
                outs=[ar_out.opt()],
            )
            # readback as [d, (r h c)]; fold r then h with two DVE adds.
            nc.sync.dma_start(
                rb[:].rearrange("d (r h c) -> d r h c", r=2, h=2),
                ar_out.rearrange("r (h d) c -> d r h c", h=2),
            )
            nc.vector.tensor_add(rb[:, 0:2 * C], rb[:, 0:2 * C], rb[:, 2 * C:4 * C])
            nc.vector.tensor_add(kvt_red[:], rb[:, 0:C], rb[:, C:2 * C])

            # ---- softmax over c (free axis) on kvT ----
            neg_mx = tmp.tile([C2, 1], F32)
            nc.vector.reduce_max(
                out=neg_mx[:],
                in_=kvt_red[:],
                axis=mybir.AxisListType.X,
                negate=True,
            )
            ex = tmp.tile([C2, C], F32)
            sm = tmp.tile([C2, 1], F32)
            nc.scalar.activation(
                ex[:],
                kvt_red[:],
                mybir.ActivationFunctionType.Exp,
                bias=neg_mx[:], scale=1.0,
                accum_out=sm[:],
            )
            rv = tmp.tile([C2, 1], F32)
            nc.vector.reciprocal(rv[:], sm[:])
            nc.vector.tensor_scalar_mul(ex[:], ex[:], rv[:])

            # transpose softmaxed kvT -> kv[c, d], cast into W rows 0:64
            tp = ps.tile([C, C2], F32, tag="tp", name="tp")
            nc.tensor.transpose(tp[:], ex[:], ident[0:C2, 0:C2])
            nc.vector.tensor_copy(w_sb[0:C, :], tp[:])

            # ---- phase 2: outT = W^T @ [key_curT; val_curT], fused add.
            # Col group 0 -> psum rows 0:64 (token group A), col group 64
            # -> rows 64:128 (group B). PE can't be DMAd directly, so the
            # PSUM -> SBUF moves round-robin over DVE/GpSimd/ACT to run
            # concurrently; one store per 2x512-token tile.
            stg = persist.tile([128, NSL // 2], F32)
            HS = NSL // 2
            W = 512
            for g in range(G_TILES):
                o = po.tile([128, W], F32, tag="o", name=f"o{g}")
                colA = slice(g * W, (g + 1) * W)
                colB = slice(HS + g * W, HS + (g + 1) * W)
                nc.tensor.matmul(
                    o[0:C2, :],
                    lhsT=w_sb[:],
                    rhs=kvc_sb[:, colA],
                    start=True, stop=True,
                    tile_position=(0, 0),
                )
                nc.tensor.matmul(
                    o[64:64 + C2, :],
                    lhsT=w_sb[:],
                    rhs=kvc_sb[:, colB],
                    start=True, stop=True,
                    tile_position=(0, 64),
                )
                if g % 2 == 1:
                    nc.scalar.activation(
                        stg[:, colA], o[:], mybir.ActivationFunctionType.Copy
                    )
                else:
                    nc.vector.tensor_copy(stg[:, colA], o[:])
                nc.sync.dma_start(out[:, colA], stg[:, colA])

    nc.compile()
    return nc


def _get_program():
    if "nc" not in _CACHE:
        _CACHE["nc"] = _build_program()
    return _CACHE["nc"]


def kernel(key_mem, val_mem, key_cur, val_cur, alpha):
    key_mem = np.asarray(key_mem, dtype=np.float32)
    val_mem = np.asarray(val_mem, dtype=np.float32)
    key_cur = np.asarray(key_cur, dtype=np.float32)
    val_cur = np.asarray(val_cur, dtype=np.float32)
    alpha_f = float(np.asarray(alpha).reshape(-1)[0])

    nc = _get_program()

    km_flat = key_mem.reshape(N, TOK_ALL, C).astype(np.float16)
    vm_flat = val_mem.reshape(N, TOK_ALL, C2).astype(np.float16)
    kc_scaled = (alpha_f * key_cur).astype(np.float16)
    vc16 = val_cur.astype(np.float16)

    in_maps = []
    for core in range(N_CORES):
        b, h = divmod(core, 2)
        s0 = h * NSL
        kvc_i = np.empty((128, NSL), dtype=np.float16)
        kvc_i[0:64, 0:NSL // 2] = kc_scaled[b, s0:s0 + NSL // 2, :].T
        kvc_i[0:64, NSL // 2:] = kc_scaled[b, s0 + NSL // 2:s0 + NSL, :].T
        kvc_i[64:128, 0:NSL // 2] = vc16[b, s0:s0 + NSL // 2, :].T
        kvc_i[64:128, NSL // 2:] = vc16[b, s0 + NSL // 2:s0 + NSL, :].T
        in_maps.append(
            {
                "key_mem": np.ascontiguousarray(
                    km_flat[b, h * TOK_HALF:(h + 1) * TOK_HALF]
                ),
                "val_mem": np.ascontiguousarray(
                    vm_flat[b, h * TOK_HALF:(h + 1) * TOK_HALF]
                ),
                "kvc": kvc_i,
            }
        )

    res = bass_utils.run_bass_kernel_spmd(
        nc, in_maps, core_ids=list(range(N_CORES)), **_RUN_OPTS
    )
    _CACHE["last_result"] = res

    out = np.empty((N, NTOK, C2), dtype=np.float32)
    for core in range(N_CORES):
        b, h = divmod(core, 2)
        s0 = h * NSL
        o = res.results[core]["outT"]
        out[b, s0:s0 + NSL // 2, :] = o[0:64, :].T
        out[b, s0 + NSL // 2:s0 + NSL, :] = o[64:128, :].T
    return out


# revision 12
# speedup vs baseline: 1.0232x; 1.0232x over previous
"""ChannelAttentionPropagation1D kernel for 8x TRN2 NeuronCores.

Reference computation (per batch b):
  kv[c,d]   = sum_{t,n} key_mem[b,t,n,c] * val_mem[b,t,n,d]    # (64, 64)
  kv_soft   = softmax(kv, axis=c)
  out[n,d]  = alpha * (key_cur[b] @ kv_soft)[n,d] + val_cur[b,n,d]

Sharding (8 cores): batch-pair. Core c owns batch b = c//2, token-half
h = c%2. Phase 1 contracts its 65536-token half of key_mem/val_mem into
a partial kvT; ONE 2-rank AllGather (cores 2b <-> 2b+1) exchanges the
32KB PSUM partials; both cores reduce + softmax locally. Phase 2
computes the core's 8192-token slice of the output.

Precision: the kv softmax is extremely sharp (top-2 logit gap ~500) and
alpha is small, so fp16 inputs to both matmuls keep the final rel-fro
error ~2e-4, far under the 2e-2 gate, while halving HBM traffic. The
host casts all inputs to fp16; PSUM accumulation stays fp32 and the
output is stored fp32.

Layouts:
  - phase 1 accumulates kvT[d,c] in PSUM with two alternating PE column
    groups; the raw [128,64] PSUM partial is DMAd straight to the
    collective input (no pre-reduction), and the fold happens after the
    gather with two DVE adds.
  - phase 2 is computed TRANSPOSED and FUSED: the stationary operand is
    W = [kv_soft[c,d]; I64], and the moving operand stacks
    [alpha*key_curT; val_curT] on the 128 partitions, so one matmul per
    512-token tile yields outT[d,tok] = kv_soft^T@key_curT + val_curT
    directly in PSUM (no DVE adds); stores go PSUM -> DRAM. Two PE
    column groups process the core's two 4096-token groups concurrently.
    The host transposes the returned [128, 4096] block back to
    [8192, 64].
"""

import numpy as np

import concourse.bacc as bacc
import concourse.mybir as mybir
import concourse.tile as tile
from concourse import bass_utils, masks

F32 = mybir.dt.float32
F16 = mybir.dt.float16

N_CORES = 8
N, T, NTOK, C, C2 = 4, 8, 16384, 64, 64
TOK_ALL = T * NTOK          # 131072 tokens per batch
TOK_HALF = TOK_ALL // 2     # 65536 phase-1 tokens per core
NSL = NTOK // 2             # 8192 phase-2 output tokens per core
CHUNK = 8192                # phase-1 tokens per DMA chunk
N_CHUNKS = TOK_HALF // CHUNK    # 8
A_TILES = CHUNK // 128      # 64 matmul token-tiles per chunk
G_TILES = 8                 # phase-2: 8 psum tiles of 2x512 tokens

_CACHE = {}

# Extra kwargs forwarded to run_bass_kernel_spmd (used by the profiling
# harness to request an NTFF trace; empty for normal correctness runs).
_RUN_OPTS = {}


def _build_program():
    nc = bacc.Bacc(
        "TRN2",
        target_bir_lowering=False,
        debug=False,
        enable_asserts=False,
        num_devices=N_CORES,
    )

    km = nc.dram_tensor("key_mem", [TOK_HALF, C], F16, kind="ExternalInput").ap()
    vm = nc.dram_tensor("val_mem", [TOK_HALF, C2], F16, kind="ExternalInput").ap()
    # kvc: [128, 8192] fp16. Rows 0:64 = alpha*key_cur^T (channels c),
    # rows 64:128 = val_cur^T (channels d). Cols 0:4096 = token group A
    # (slice tokens 0:4096), cols 4096:8192 = group B (4096:8192).
    kvc = nc.dram_tensor("kvc", [128, NSL], F16, kind="ExternalInput").ap()
    # outT: rows 0:64 = out[d, tok] for group A, rows 64:128 for group B.
    out = nc.dram_tensor("outT", [128, NSL // 2], F32, kind="ExternalOutput").ap()

    with tile.TileContext(nc) as tc:
        with (
            tc.tile_pool(name="persist", bufs=1) as persist,
            tc.tile_pool(name="big", bufs=4) as big,
            tc.tile_pool(name="tmp", bufs=1) as tmp,
            tc.tile_pool(name="ps", bufs=1, space="PSUM") as ps,
            tc.tile_pool(name="po", bufs=4, space="PSUM") as po,
            tc.tile_pool(name="dram", bufs=1, space="DRAM") as dram,
        ):
            ident = persist.tile([128, 128], F32)
            masks.make_identity(nc, ident[:])

            # Warm-up collective on junk data, triggered immediately: the
            # FIRST data op on the CC stream pays ~11.5us of one-time setup
            # (baseline traces: op0 start delay 11.5us, later ops ~1.5us).
            # Paying it here hides it under the phase-1 load stream.
            wu_in = dram.tile([64, 1], F32, tag="wu_in", name="wu_in")
            wu_out = dram.tile([2, 64, 1], F32, tag="wu_out", name="wu_out")
            nc.gpsimd.collective_compute(
                "AllGather",
                mybir.AluOpType.bypass,
                replica_groups=[[2 * i, 2 * i + 1] for i in range(4)],
                ins=[wu_in.opt()],
                outs=[wu_out.opt()],
            )

            kvc_sb = persist.tile([128, NSL], F16)
            # stationary phase-2 weights: rows 0:64 = kv_soft[c, d],
            # rows 64:128 = I64 (the val_cur passthrough).
            w_sb = persist.tile([128, C2], F16)
            nc.vector.tensor_copy(w_sb[64:128, :], ident[0:64, 0:64])

            kvt_red = persist.tile([C2, C], F32)

            # ---- phase 1: partial kvT[d, c], col-tiled 2x ----
            kv_ps = ps.tile([128, C], F32)
            for ch in range(N_CHUNKS):
                k_sb = big.tile([128, CHUNK // 128 * C], F16, tag="k")
                v_sb = big.tile([128, CHUNK // 128 * C2], F16, tag="v")
                sl = slice(ch * CHUNK, (ch + 1) * CHUNK)
                nc.sync.dma_start(
                    k_sb[:], km[sl, :].rearrange("(p a) c -> p (a c)", p=128)
                )
                nc.sync.dma_start(
                    v_sb[:], vm[sl, :].rearrange("(p a) c -> p (a c)", p=128)
                )
                if ch == 1:
                    # phase-2 inputs ride the scalar (ACT) DMA FIFO so they
                    # never delay the phase-1 chunk stream on the sync FIFO.
                    nc.scalar.dma_start(kvc_sb[:], kvc)
                for a in range(A_TILES):
                    half = a % 2
                    nc.tensor.matmul(
                        kv_ps[64 * half:64 * half + C2, :],
                        lhsT=v_sb[:, a * C2:(a + 1) * C2],
                        rhs=k_sb[:, a * C:(a + 1) * C],
                        start=(ch == 0 and a < 2),
                        stop=(ch == N_CHUNKS - 1 and a >= A_TILES - 2),
                        tile_position=(0, 64 * half),
                    )

            # ---- pairwise exchange: 2-rank AllGather of the pre-reduced
            # fp16 partial (8KB; the CC is latency-bound, so smaller is
            # faster: ~0.44ns/B slope measured on this stream).
            kvt16 = persist.tile([C2, C], F16)
            nc.vector.tensor_copy(kvt_red[:], kv_ps[0:C2, :])
            nc.vector.tensor_add(kvt_red[:], kvt_red[:], kv_ps[64:64 + C2, :])
            nc.vector.tensor_copy(kvt16[:], kvt_red[:])
            ar_in = dram.tile([C2, C], F16, tag="ar_in", name="ar_in")
            ar_out = dram.tile([2, C2, C], F16, tag="ar_out", name="ar_out")
            nc.scalar.dma_start(ar_in[:], kvt16[:])
            nc.gpsimd.collective_compute(
                "AllGather",
                mybir.AluOpType.bypass,
                replica_groups=[[2 * i, 2 * i + 1] for i in range(4)],
                ins=[ar_in.opt()],
                outs=[ar_out.opt()],
            )
            # readback as [d, (r c)]; fold r with one DVE add (fp16 in,
            # fp32 out).
            rb16 = persist.tile([C2, 2 * C], F16)
            nc.sync.dma_start(
                rb16[:].rearrange("d (r c) -> d r c", r=2),
                ar_out.rearrange("r d c -> d r c"),
            )
            nc.vector.tensor_add(kvt_red[:], rb16[:, 0:C], rb16[:, C:2 * C])

            # ---- softmax over c (free axis) on kvT ----
            neg_mx = tmp.tile([C2, 1], F32)
            nc.vector.reduce_max(
                out=neg_mx[:],
                in_=kvt_red[:],
                axis=mybir.AxisListType.X,
                negate=True,
            )
            ex = tmp.tile([C2, C], F32)
            sm = tmp.tile([C2, 1], F32)
            nc.scalar.activation(
                ex[:],
                kvt_red[:],
                mybir.ActivationFunctionType.Exp,
                bias=neg_mx[:], scale=1.0,
                accum_out=sm[:],
            )
            rv = tmp.tile([C2, 1], F32)
            nc.vector.reciprocal(rv[:], sm[:])
            nc.vector.tensor_scalar_mul(ex[:], ex[:], rv[:])

            # transpose softmaxed kvT -> kv[c, d], cast into W rows 0:64
            tp = ps.tile([C, C2], F32, tag="tp", name="tp")
            nc.tensor.transpose(tp[:], ex[:], ident[0:C2, 0:C2])
            nc.vector.tensor_copy(w_sb[0:C, :], tp[:])

            # ---- phase 2: outT = W^T @ [key_curT; val_curT], fused add.
            # Col group 0 -> psum rows 0:64 (token group A), col group 64
            # -> rows 64:128 (group B). PE can't be DMAd directly, so the
            # PSUM -> SBUF moves round-robin over DVE/GpSimd/ACT to run
            # concurrently; one store per 2x512-token tile.
            stg = persist.tile([128, NSL // 2], F32)
            HS = NSL // 2
            W = 512
            for g in range(G_TILES):
                o = po.tile([128, W], F32, tag="o", name=f"o{g}")
                colA = slice(g * W, (g + 1) * W)
                colB = slice(HS + g * W, HS + (g + 1) * W)
                nc.tensor.matmul(
                    o[0:C2, :],
                    lhsT=w_sb[:],
                    rhs=kvc_sb[:, colA],
                    start=True, stop=True,
                    tile_position=(0, 0),
                )
                nc.tensor.matmul(
                    o[64:64 + C2, :],
                    lhsT=w_sb[:],
                    rhs=kvc_sb[:, colB],
                    start=True, stop=True,
                    tile_position=(0, 64),
                )
                if g % 2 == 1:
                    nc.scalar.activation(
                        stg[:, colA], o[:], mybir.ActivationFunctionType.Copy
                    )
                else:
                    nc.vector.tensor_copy(stg[:, colA], o[:])
                if g % 2 == 1:
                    st = slice((g - 1) * W, (g + 1) * W)
                    nc.sync.dma_start(out[:, st], stg[:, st])

    nc.compile()
    return nc


def _get_program():
    if "nc" not in _CACHE:
        _CACHE["nc"] = _build_program()
    return _CACHE["nc"]


def kernel(key_mem, val_mem, key_cur, val_cur, alpha):
    key_mem = np.asarray(key_mem, dtype=np.float32)
    val_mem = np.asarray(val_mem, dtype=np.float32)
    key_cur = np.asarray(key_cur, dtype=np.float32)
    val_cur = np.asarray(val_cur, dtype=np.float32)
    alpha_f = float(np.asarray(alpha).reshape(-1)[0])

    nc = _get_program()

    km_flat = key_mem.reshape(N, TOK_ALL, C).astype(np.float16)
    vm_flat = val_mem.reshape(N, TOK_ALL, C2).astype(np.float16)
    kc_scaled = (alpha_f * key_cur).astype(np.float16)
    vc16 = val_cur.astype(np.float16)

    in_maps = []
    for core in range(N_CORES):
        b, h = divmod(core, 2)
        s0 = h * NSL
        kvc_i = np.empty((128, NSL), dtype=np.float16)
        kvc_i[0:64, 0:NSL // 2] = kc_scaled[b, s0:s0 + NSL // 2, :].T
        kvc_i[0:64, NSL // 2:] = kc_scaled[b, s0 + NSL // 2:s0 + NSL, :].T
        kvc_i[64:128, 0:NSL // 2] = vc16[b, s0:s0 + NSL // 2, :].T
        kvc_i[64:128, NSL // 2:] = vc16[b, s0 + NSL // 2:s0 + NSL, :].T
        in_maps.append(
            {
                "key_mem": np.ascontiguousarray(
                    km_flat[b, h * TOK_HALF:(h + 1) * TOK_HALF]
                ),
                "val_mem": np.ascontiguousarray(
                    vm_flat[b, h * TOK_HALF:(h + 1) * TOK_HALF]
                ),
                "kvc": kvc_i,
            }
        )

    res = bass_utils.run_bass_kernel_spmd(
        nc, in_maps, core_ids=list(range(N_CORES)), **_RUN_OPTS
    )
    _CACHE["last_result"] = res

    out = np.empty((N, NTOK, C2), dtype=np.float32)
    for core in range(N_CORES):
        b, h = divmod(core, 2)
        s0 = h * NSL
        o = res.results[core]["outT"]
        out[b, s0:s0 + NSL // 2, :] = o[0:64, :].T
        out[b, s0 + NSL // 2:s0 + NSL, :] = o[64:128, :].T
    return out


# revision 16
# speedup vs baseline: 1.0734x; 1.0491x over previous
"""ChannelAttentionPropagation1D kernel for 8x TRN2 NeuronCores.

Reference computation (per batch b):
  kv[c,d]   = sum_{t,n} key_mem[b,t,n,c] * val_mem[b,t,n,d]    # (64, 64)
  kv_soft   = softmax(kv, axis=c)
  out[n,d]  = alpha * (key_cur[b] @ kv_soft)[n,d] + val_cur[b,n,d]

Sharding (8 cores): batch-pair. Core c owns batch b = c//2, token-half
h = c%2. Phase 1 contracts its 65536-token half of key_mem/val_mem into
a partial kvT; ONE 2-rank AllGather (cores 2b <-> 2b+1) exchanges the
32KB PSUM partials; both cores reduce + softmax locally. Phase 2
computes the core's 8192-token slice of the output.

Precision: the kv softmax is extremely sharp (top-2 logit gap ~500) and
alpha is small, so fp16 inputs to both matmuls keep the final rel-fro
error ~2e-4, far under the 2e-2 gate, while halving HBM traffic. The
host casts all inputs to fp16; PSUM accumulation stays fp32 and the
output is stored fp32.

Layouts:
  - phase 1 accumulates kvT[d,c] in PSUM with two alternating PE column
    groups; the raw [128,64] PSUM partial is DMAd straight to the
    collective input (no pre-reduction), and the fold happens after the
    gather with two DVE adds.
  - phase 2 is computed TRANSPOSED and FUSED: the stationary operand is
    W = [kv_soft[c,d]; I64], and the moving operand stacks
    [alpha*key_curT; val_curT] on the 128 partitions, so one matmul per
    512-token tile yields outT[d,tok] = kv_soft^T@key_curT + val_curT
    directly in PSUM (no DVE adds); stores go PSUM -> DRAM. Two PE
    column groups process the core's two 4096-token groups concurrently.
    The host transposes the returned [128, 4096] block back to
    [8192, 64].
"""

import numpy as np

import concourse.bacc as bacc
import concourse.mybir as mybir
import concourse.tile as tile
from concourse import bass_utils, masks

F32 = mybir.dt.float32
F16 = mybir.dt.float16

N_CORES = 8
N, T, NTOK, C, C2 = 4, 8, 16384, 64, 64
TOK_ALL = T * NTOK          # 131072 tokens per batch
TOK_HALF = TOK_ALL // 2     # 65536 phase-1 tokens per core
NSL = NTOK // 2             # 8192 phase-2 output tokens per core
CHUNK = 8192                # phase-1 tokens per DMA chunk
N_CHUNKS = TOK_HALF // CHUNK    # 8
A_TILES = CHUNK // 128      # 64 matmul token-tiles per chunk
G_TILES = 8                 # phase-2: 8 psum tiles of 2x512 tokens

_CACHE = {}

# Extra kwargs forwarded to run_bass_kernel_spmd (used by the profiling
# harness to request an NTFF trace; empty for normal correctness runs).
_RUN_OPTS = {}


def _build_program():
    nc = bacc.Bacc(
        "TRN2",
        target_bir_lowering=False,
        debug=False,
        enable_asserts=False,
        num_devices=N_CORES,
    )

    km = nc.dram_tensor("key_mem", [TOK_HALF, C], F16, kind="ExternalInput").ap()
    vm = nc.dram_tensor("val_mem", [TOK_HALF, C2], F16, kind="ExternalInput").ap()
    # kvc: [128, 8192] fp16. Rows 0:64 = alpha*key_cur^T (channels c),
    # rows 64:128 = val_cur^T (channels d). Cols 0:4096 = token group A
    # (slice tokens 0:4096), cols 4096:8192 = group B (4096:8192).
    kvc = nc.dram_tensor("kvc", [128, NSL], F16, kind="ExternalInput").ap()
    # outT: rows 0:64 = out[d, tok] for group A, rows 64:128 for group B.
    # fp16 on the wire (host upcasts); values pass the 2e-2 gate with
    # ~10x margin and the halved store + single DMA shortens the
    # semaphore teardown chain.
    out = nc.dram_tensor("outT", [128, NSL // 2], F16, kind="ExternalOutput").ap()

    with tile.TileContext(nc) as tc:
        with (
            tc.tile_pool(name="persist", bufs=1) as persist,
            tc.tile_pool(name="big", bufs=4) as big,
            tc.tile_pool(name="tmp", bufs=1) as tmp,
            tc.tile_pool(name="ps", bufs=1, space="PSUM") as ps,
            tc.tile_pool(name="po", bufs=4, space="PSUM") as po,
            tc.tile_pool(name="dram", bufs=1, space="DRAM") as dram,
        ):
            ident = persist.tile([128, 128], F32)
            masks.make_identity(nc, ident[:])

            # Warm-up collective on junk data, triggered immediately: the
            # FIRST data op on the CC stream pays ~11.5us of one-time setup
            # (baseline traces: op0 start delay 11.5us, later ops ~1.5us).
            # Paying it here hides it under the phase-1 load stream.
            wu_in = dram.tile([64, 1], F32, tag="wu_in", name="wu_in")
            wu_out = dram.tile([2, 64, 1], F32, tag="wu_out", name="wu_out")
            nc.gpsimd.collective_compute(
                "AllGather",
                mybir.AluOpType.bypass,
                replica_groups=[[2 * i, 2 * i + 1] for i in range(4)],
                ins=[wu_in.opt()],
                outs=[wu_out.opt()],
            )

            kvc_sb = persist.tile([128, NSL], F16)
            # stationary phase-2 weights: rows 0:64 = kv_soft[c, d],
            # rows 64:128 = I64 (the val_cur passthrough).
            w_sb = persist.tile([128, C2], F16)
            nc.vector.tensor_copy(w_sb[64:128, :], ident[0:64, 0:64])

            kvt_red = persist.tile([C2, C], F32)

            # ---- phase 1: partial kvT[d, c], col-tiled 2x ----
            kv_ps = ps.tile([128, C], F32)
            for ch in range(N_CHUNKS):
                k_sb = big.tile([128, CHUNK // 128 * C], F16, tag="k")
                v_sb = big.tile([128, CHUNK // 128 * C2], F16, tag="v")
                sl = slice(ch * CHUNK, (ch + 1) * CHUNK)
                nc.sync.dma_start(
                    k_sb[:], km[sl, :].rearrange("(p a) c -> p (a c)", p=128)
                )
                nc.sync.dma_start(
                    v_sb[:], vm[sl, :].rearrange("(p a) c -> p (a c)", p=128)
                )
                if ch == 1:
                    # phase-2 inputs ride the scalar (ACT) DMA FIFO so they
                    # never delay the phase-1 chunk stream on the sync FIFO.
                    nc.scalar.dma_start(kvc_sb[:], kvc)
                for a in range(A_TILES):
                    half = a % 2
                    nc.tensor.matmul(
                        kv_ps[64 * half:64 * half + C2, :],
                        lhsT=v_sb[:, a * C2:(a + 1) * C2],
                        rhs=k_sb[:, a * C:(a + 1) * C],
                        start=(ch == 0 and a < 2),
                        stop=(ch == N_CHUNKS - 1 and a >= A_TILES - 2),
                        tile_position=(0, 64 * half),
                    )

            # ---- pairwise exchange: 2-rank AllGather of the pre-reduced
            # fp16 partial (8KB; the CC is latency-bound, so smaller is
            # faster: ~0.44ns/B slope measured on this stream).
            kvt16 = persist.tile([C2, C], F16)
            nc.vector.tensor_copy(kvt_red[:], kv_ps[0:C2, :])
            nc.vector.tensor_add(kvt_red[:], kvt_red[:], kv_ps[64:64 + C2, :])
            nc.vector.tensor_copy(kvt16[:], kvt_red[:])
            ar_in = dram.tile([C2, C], F16, tag="ar_in", name="ar_in")
            ar_out = dram.tile([2, C2, C], F16, tag="ar_out", name="ar_out")
            nc.scalar.dma_start(ar_in[:], kvt16[:])
            nc.gpsimd.collective_compute(
                "AllGather",
                mybir.AluOpType.bypass,
                replica_groups=[[2 * i, 2 * i + 1] for i in range(4)],
                ins=[ar_in.opt()],
                outs=[ar_out.opt()],
            )
            # readback as [d, (r c)]; fold r with one DVE add (fp16 in,
            # fp32 out).
            rb16 = persist.tile([C2, 2 * C], F16)
            nc.sync.dma_start(
                rb16[:].rearrange("d (r c) -> d r c", r=2),
                ar_out.rearrange("r d c -> d r c"),
            )
            nc.vector.tensor_add(kvt_red[:], rb16[:, 0:C], rb16[:, C:2 * C])

            # ---- softmax over c (free axis) on kvT ----
            neg_mx = tmp.tile([C2, 1], F32)
            nc.vector.reduce_max(
                out=neg_mx[:],
                in_=kvt_red[:],
                axis=mybir.AxisListType.X,
                negate=True,
            )
            ex = tmp.tile([C2, C], F32)
            sm = tmp.tile([C2, 1], F32)
            nc.scalar.activation(
                ex[:],
                kvt_red[:],
                mybir.ActivationFunctionType.Exp,
                bias=neg_mx[:], scale=1.0,
                accum_out=sm[:],
            )
            rv = tmp.tile([C2, 1], F32)
            nc.vector.reciprocal(rv[:], sm[:])
            nc.vector.tensor_scalar_mul(ex[:], ex[:], rv[:])

            # transpose softmaxed kvT -> kv[c, d], cast into W rows 0:64
            tp = ps.tile([C, C2], F32, tag="tp", name="tp")
            nc.tensor.transpose(tp[:], ex[:], ident[0:C2, 0:C2])
            nc.vector.tensor_copy(w_sb[0:C, :], tp[:])

            # ---- phase 2: outT = W^T @ [key_curT; val_curT], fused add.
            # Col group 0 -> psum rows 0:64 (token group A), col group 64
            # -> rows 64:128 (group B). PE can't be DMAd directly, so the
            # PSUM -> SBUF moves round-robin over DVE/GpSimd/ACT to run
            # concurrently; one store per 2x512-token tile.
            stg = persist.tile([128, NSL // 2], F16)
            HS = NSL // 2
            W = 512
            for g in range(G_TILES):
                o = po.tile([128, W], F32, tag="o", name=f"o{g}")
                colA = slice(g * W, (g + 1) * W)
                colB = slice(HS + g * W, HS + (g + 1) * W)
                nc.tensor.matmul(
                    o[0:C2, :],
                    lhsT=w_sb[:],
                    rhs=kvc_sb[:, colA],
                    start=True, stop=True,
                    tile_position=(0, 0),
                )
                nc.tensor.matmul(
                    o[64:64 + C2, :],
                    lhsT=w_sb[:],
                    rhs=kvc_sb[:, colB],
                    start=True, stop=True,
                    tile_position=(0, 64),
                )
                if g % 2 == 1:
                    nc.scalar.activation(
                        stg[:, colA], o[:], mybir.ActivationFunctionType.Copy
                    )
                else:
                    nc.vector.tensor_copy(stg[:, colA], o[:])
            nc.sync.dma_start(out[:], stg[:])

    nc.compile()
    return nc


def _get_program():
    if "nc" not in _CACHE:
        _CACHE["nc"] = _build_program()
    return _CACHE["nc"]


def kernel(key_mem, val_mem, key_cur, val_cur, alpha):
    key_mem = np.asarray(key_mem, dtype=np.float32)
    val_mem = np.asarray(val_mem, dtype=np.float32)
    key_cur = np.asarray(key_cur, dtype=np.float32)
    val_cur = np.asarray(val_cur, dtype=np.float32)
    alpha_f = float(np.asarray(alpha).reshape(-1)[0])

    nc = _get_program()

    km_flat = key_mem.reshape(N, TOK_ALL, C).astype(np.float16)
    vm_flat = val_mem.reshape(N, TOK_ALL, C2).astype(np.float16)
    kc_scaled = (alpha_f * key_cur).astype(np.float16)
    vc16 = val_cur.astype(np.float16)

    in_maps = []
    for core in range(N_CORES):
        b, h = divmod(core, 2)
        s0 = h * NSL
        kvc_i = np.empty((128, NSL), dtype=np.float16)
        kvc_i[0:64, 0:NSL // 2] = kc_scaled[b, s0:s0 + NSL // 2, :].T
        kvc_i[0:64, NSL // 2:] = kc_scaled[b, s0 + NSL // 2:s0 + NSL, :].T
        kvc_i[64:128, 0:NSL // 2] = vc16[b, s0:s0 + NSL // 2, :].T
        kvc_i[64:128, NSL // 2:] = vc16[b, s0 + NSL // 2:s0 + NSL, :].T
        in_maps.append(
            {
                "key_mem": np.ascontiguousarray(
                    km_flat[b, h * TOK_HALF:(h + 1) * TOK_HALF]
                ),
                "val_mem": np.ascontiguousarray(
                    vm_flat[b, h * TOK_HALF:(h + 1) * TOK_HALF]
                ),
                "kvc": kvc_i,
            }
        )

    res = bass_utils.run_bass_kernel_spmd(
        nc, in_maps, core_ids=list(range(N_CORES)), **_RUN_OPTS
    )
    _CACHE["last_result"] = res

    out = np.empty((N, NTOK, C2), dtype=np.float32)
    for core in range(N_CORES):
        b, h = divmod(core, 2)
        s0 = h * NSL
        o = res.results[core]["outT"].astype(np.float32)
        out[b, s0:s0 + NSL // 2, :] = o[0:64, :].T
        out[b, s0 + NSL // 2:s0 + NSL, :] = o[64:128, :].T
    return out
